# revision 1
# baseline (speedup 1.0000x reference)
"""GPT-2 small (L=12, D=768, H=12, S=1024, B=2, V=50257) forward pass on 8
Trainium2 NeuronCores via Bass/Tile.

Sharding: data-parallel over batch + vocab-parallel head, zero collectives.
Measured AllReduce cost on this runtime is ~150-250us fixed per call, so any
per-layer collective scheme (24 calls) loses to redundant compute. Instead:
  - cores 0-3 all compute the full 12-layer body for batch 0 (redundantly,
    SPMD-identical), cores 4-7 for batch 1
  - each core then computes its own quarter of the vocab for its batch's
    LM head (12565-ish cols/core, padded 12800) - the only sharded part
The body loops over 4 weight column-slices per layer (g-loop) accumulating
partial sums locally, which keeps every SBUF tile small.

Layout: activations are kept transposed (feature on partitions, tokens on the
free axis) so every dense matmul takes the weight straight from HBM as the
stationary lhsT with no transposes anywhere. Attention scores are computed in
[key, query] layout; softmax uses exp without max-subtraction (scores for this
model/data are bounded well inside fp32 exp range), the key-sum is a
partition_all_reduce, and 1/sum is folded into the PSUM->SBUF copy of the
attention output. Matmul inputs are float32r (full-rate PE, ~1.5e-4 rel err).
"""

import numpy as np

import concourse.bass as bass
import concourse.tile as tile
from concourse import bacc, mybir
from concourse import bass_utils
from concourse.bass_isa import ReduceOp

F32 = mybir.dt.float32
F32R = mybir.dt.float32r
AL = mybir.AluOpType
ACT = mybir.ActivationFunctionType

# model dims
B, S, D, H, DH, F4, V, L = 2, 1024, 768, 12, 64, 3072, 50257, 12
P = 128
KT = D // P            # 6 k-tiles over the model dim
EPS = 1e-5
SCALE = 1.0 / np.sqrt(DH)

# sharding
NCORES = 8
TPG = 4                # weight column-slices per layer (g loop)
HPC = H // TPG         # heads per slice
DL = HPC * DH          # slice attn width 192
FFL = F4 // TPG        # slice ffn width 768
QB = 512               # query block
NQB = S // QB
NKT = S // P           # key tiles
VC = 512               # vocab chunk
VPAD = 12800           # padded per-core vocab slice (25 chunks of 512)
NVC = VPAD // VC
VSLICE = [12565, 12564, 12564, 12564]
VSTART = [0, 12565, 25129, 37693]

L_BODY = L  # overridable before first kernel() call for debugging

_CACHE = {}


def _build():
    nc = bacc.Bacc("TRN2", target_bir_lowering=False, debug=False,
                   num_devices=NCORES)

    def di(name, shape, dt=F32):
        return nc.dram_tensor(name, shape, dt, kind="ExternalInput").ap()

    x0T = di("x0T", [D, S])
    masks = di("masks", [P, TPG, QB])
    wq_s = di("wq_s", [L_BODY, D, D], F32R)
    wk_s = di("wk_s", [L_BODY, D, D], F32R)
    wv_s = di("wv_s", [L_BODY, D, D], F32R)
    wo_s = di("wo_s", [L_BODY, D, D], F32R)
    w1_s = di("w1_s", [L_BODY, D, F4], F32R)
    w2_s = di("w2_s", [L_BODY, F4, D], F32R)
    ln1g = di("ln1g", [L_BODY, P, KT])
    ln1b = di("ln1b", [L_BODY, P, KT])
    ln2g = di("ln2g", [L_BODY, P, KT])
    ln2b = di("ln2b", [L_BODY, P, KT])
    bq_s = di("bq_s", [L_BODY, DH, H])
    bk_s = di("bk_s", [L_BODY, DH, H])
    bv_s = di("bv_s", [L_BODY, TPG, DL])
    bo_s = di("bo_s", [L_BODY, P, KT])
    b1_s = di("b1_s", [L_BODY, TPG, P, KT])
    b2_s = di("b2_s", [L_BODY, P, KT])
    fng = di("fng", [P, KT])
    fnb = di("fnb", [P, KT])
    hw_s = di("hw_s", [D, VPAD], F32R)
    hb_s = di("hb_s", [1, VPAD])
    logits = nc.dram_tensor("logits", [S, VPAD], F32, kind="ExternalOutput").ap()

    with tile.TileContext(nc) as tc:
        with tc.tile_pool(name="persist", bufs=1) as persist, \
             tc.tile_pool(name="slab", bufs=2) as slab, \
             tc.tile_pool(name="wpool", bufs=2) as wpool, \
             tc.tile_pool(name="qk", bufs=2) as qkpool, \
             tc.tile_pool(name="vp", bufs=1) as vpool, \
             tc.tile_pool(name="op", bufs=1) as opool, \
             tc.tile_pool(name="ep", bufs=3) as eppool, \
             tc.tile_pool(name="sums", bufs=2) as sums, \
             tc.tile_pool(name="tmp", bufs=3) as tmp, \
             tc.tile_pool(name="small", bufs=3) as small, \
             tc.tile_pool(name="psA", bufs=4, space="PSUM") as psA, \
             tc.tile_pool(name="psO", bufs=2, space="PSUM") as psO:

            xT = persist.tile([P, KT, S], F32)
            nc.sync.dma_start(xT, x0T.rearrange("(t p) q -> p t q", p=P))
            masks_sb = persist.tile([P, TPG, QB], F32)
            nc.sync.dma_start(masks_sb, masks)

            def layer_norm(g_ap, b_ap, out_dt=F32R):
                """LN over the feature (partition x KT) axis of xT, done per
                query block. Returns a fresh slab tile with the result."""
                g_t = small.tile([P, KT], F32, tag="gain")
                b_t = small.tile([P, KT], F32, tag="gain")
                nc.sync.dma_start(g_t, g_ap)
                nc.sync.dma_start(b_t, b_ap)
                out = slab.tile([P, KT, S], out_dt, tag="slab")
                for qb in range(NQB):
                    qs = slice(qb * QB, (qb + 1) * QB)
                    acc = tmp.tile([P, QB], F32, tag="acc")
                    accsq = tmp.tile([P, QB], F32, tag="acc")
                    sq = tmp.tile([P, QB], F32, tag="acc")
                    nc.vector.tensor_tensor(acc, xT[:, 0, qs], xT[:, 1, qs], AL.add)
                    for kt in range(2, KT):
                        nc.vector.tensor_tensor(acc, acc, xT[:, kt, qs], AL.add)
                    nc.scalar.activation(accsq, xT[:, 0, qs], ACT.Square)
                    for kt in range(1, KT):
                        nc.scalar.activation(sq, xT[:, kt, qs], ACT.Square)
                        nc.vector.tensor_tensor(accsq, accsq, sq, AL.add)
                    nc.gpsimd.partition_all_reduce(acc, acc, P, ReduceOp.add)
                    nc.gpsimd.partition_all_reduce(accsq, accsq, P, ReduceOp.add)
                    # acc -> mean; accsq -> rstd (replicated across partitions)
                    nc.vector.tensor_scalar_mul(acc, acc, 1.0 / D)
                    nc.vector.tensor_tensor(sq, acc, acc, AL.mult)
                    nc.vector.tensor_scalar_mul(accsq, accsq, 1.0 / D)
                    nc.vector.tensor_tensor(accsq, accsq, sq, AL.subtract)
                    nc.vector.tensor_scalar_add(accsq, accsq, EPS)
                    nc.scalar.activation(accsq, accsq, ACT.Sqrt)
                    nc.vector.reciprocal(accsq, accsq)
                    for kt in range(KT):
                        nc.vector.tensor_tensor(sq, xT[:, kt, qs], acc, AL.subtract)
                        nc.vector.tensor_tensor(sq, sq, accsq, AL.mult)
                        nc.vector.tensor_scalar(
                            out[:, kt, qs], sq, g_t[:, kt:kt + 1], b_t[:, kt:kt + 1],
                            AL.mult, AL.add)
                return out

            def add_residual(part, bias_ap):
                nc.vector.tensor_tensor(xT, xT, part, AL.add)
                b_t = small.tile([P, KT], F32, tag="gain")
                nc.sync.dma_start(b_t, bias_ap)
                for kt in range(KT):
                    nc.vector.tensor_scalar_add(
                        xT[:, kt, :], xT[:, kt, :], b_t[:, kt:kt + 1])

            def accum(dst_ap, ps, first):
                if first:
                    nc.vector.tensor_copy(dst_ap, ps)
                else:
                    nc.vector.tensor_tensor(dst_ap, dst_ap, ps, AL.add)

            for l in range(L_BODY):
                # ---- attention ----
                hT = layer_norm(ln1g[l], ln1b[l])
                part = slab.tile([P, KT, S], F32, tag="slab")

                for g in range(TPG):
                    cs = slice(g * DL, (g + 1) * DL)
                    wq_t = wpool.tile([P, KT, DL], F32R, tag="w")
                    nc.sync.dma_start(
                        wq_t, wq_s[l][:, cs].rearrange("(t p) f -> p t f", p=P))
                    wk_t = wpool.tile([P, KT, DL], F32R, tag="w")
                    nc.sync.dma_start(
                        wk_t, wk_s[l][:, cs].rearrange("(t p) f -> p t f", p=P))
                    wv_t = wpool.tile([P, KT, DL], F32R, tag="w")
                    nc.sync.dma_start(
                        wv_t, wv_s[l][:, cs].rearrange("(t p) f -> p t f", p=P))

                    bq_t = small.tile([DH, HPC], F32, tag="bqk")
                    bk_t = small.tile([DH, HPC], F32, tag="bqk")
                    nc.sync.dma_start(bq_t, bq_s[l][:, g * HPC:(g + 1) * HPC])
                    nc.sync.dma_start(bk_t, bk_s[l][:, g * HPC:(g + 1) * HPC])
                    bv_row = small.tile([1, DL], F32, tag="bvr")
                    nc.sync.dma_start(bv_row, bv_s[l, g:g + 1, :])
                    bv_b = small.tile([P, DL], F32, tag="bvb")
                    nc.gpsimd.partition_broadcast(bv_b, bv_row)

                    qT = qkpool.tile([DH, HPC, S], F32R, tag="qk")
                    kTt = qkpool.tile([DH, HPC, S], F32R, tag="qk")
                    for h in range(HPC):
                        for qb in range(NQB):
                            qs = slice(qb * QB, (qb + 1) * QB)
                            q_ps = psO.tile([DH, QB], F32, tag="psO")
                            k_ps = psO.tile([DH, QB], F32, tag="psO")
                            for kt in range(KT):
                                nc.tensor.matmul(
                                    q_ps, wq_t[:, kt, h * DH:(h + 1) * DH],
                                    hT[:, kt, qs], start=kt == 0, stop=kt == KT - 1)
                            nc.vector.tensor_scalar(
                                qT[:, h, qs], q_ps, bq_t[:, h:h + 1], SCALE,
                                AL.add, AL.mult)
                            for kt in range(KT):
                                nc.tensor.matmul(
                                    k_ps, wk_t[:, kt, h * DH:(h + 1) * DH],
                                    hT[:, kt, qs], start=kt == 0, stop=kt == KT - 1)
                            nc.vector.tensor_scalar_add(
                                kTt[:, h, qs], k_ps, bk_t[:, h:h + 1])
                    v_t = vpool.tile([P, NKT, DL], F32R, tag="v")
                    for tc_ in range(NKT):
                        v_ps = psA.tile([P, QB], F32, tag="psA")
                        for kt in range(KT):
                            nc.tensor.matmul(
                                v_ps[:, :DL], hT[:, kt, tc_ * P:(tc_ + 1) * P],
                                wv_t[:, kt, :], start=kt == 0, stop=kt == KT - 1)
                        nc.vector.tensor_tensor(
                            v_t[:, tc_, :], v_ps[:, :DL], bv_b, AL.add)

                    oT = opool.tile([P, 2, S], F32R, tag="o")
                    for h in range(HPC):
                        for qb in range(NQB):
                            qs = slice(qb * QB, (qb + 1) * QB)
                            nkt = 4 * qb + 4
                            o_ps = psO.tile([DH, QB], F32, tag="psO")
                            ssum = sums.tile([P, QB], F32, tag="ssum")
                            for ti in range(nkt):
                                s_ps = psA.tile([P, QB], F32, tag="psA")
                                nc.tensor.matmul(
                                    s_ps, kTt[:, h, ti * P:(ti + 1) * P],
                                    qT[:, h, qs], start=True, stop=True)
                                e_t = eppool.tile([P, QB], F32R, tag="e")
                                nc.scalar.activation(e_t, s_ps, ACT.Exp)
                                r = ti - 4 * qb
                                if r >= 0:
                                    nc.vector.tensor_tensor(
                                        e_t, e_t, masks_sb[:, r, :], AL.mult)
                                if ti == 0:
                                    nc.vector.tensor_copy(ssum, e_t)
                                else:
                                    nc.vector.tensor_tensor(ssum, ssum, e_t, AL.add)
                                nc.tensor.matmul(
                                    o_ps, v_t[:, ti, h * DH:(h + 1) * DH], e_t,
                                    start=ti == 0, stop=ti == nkt - 1)
                            nc.gpsimd.partition_all_reduce(
                                ssum, ssum, P, ReduceOp.add)
                            rcp = sums.tile([DH, QB], F32, tag="rcp")
                            nc.vector.reciprocal(rcp, ssum[:DH, :])
                            nc.vector.tensor_tensor(
                                oT[(h % 2) * DH:(h % 2) * DH + DH, h // 2, qs],
                                o_ps, rcp, AL.mult)

                    wo_t = wpool.tile([P, 2, D], F32R, tag="w")
                    r0 = g * DL
                    nc.sync.dma_start(wo_t[:, 0, :], wo_s[l, r0:r0 + P, :])
                    nc.sync.dma_start(wo_t[0:DL - P, 1, :], wo_s[l, r0 + P:r0 + DL, :])
                    for oc in range(KT):
                        for qb in range(NQB):
                            ps = psA.tile([P, QB], F32, tag="psA")
                            nc.tensor.matmul(
                                ps, wo_t[:, 0, oc * P:(oc + 1) * P],
                                oT[:, 0, qb * QB:(qb + 1) * QB],
                                start=True, stop=False)
                            nc.tensor.matmul(
                                ps, wo_t[0:DH, 1, oc * P:(oc + 1) * P],
                                oT[0:DH, 1, qb * QB:(qb + 1) * QB],
                                start=False, stop=True)
                            accum(part[:, oc, qb * QB:(qb + 1) * QB], ps, g == 0)
                add_residual(part, bo_s[l])

                # ---- ffn ----
                h2T = layer_norm(ln2g[l], ln2b[l])
                part2 = slab.tile([P, KT, S], F32, tag="slab")
                for g in range(TPG):
                    fs = slice(g * FFL, (g + 1) * FFL)
                    w1_t = wpool.tile([P, KT, FFL], F32R, tag="w")
                    nc.sync.dma_start(
                        w1_t, w1_s[l][:, fs].rearrange("(t p) f -> p t f", p=P))
                    w2_t = wpool.tile([P, KT, D], F32R, tag="w")
                    nc.sync.dma_start(
                        w2_t, w2_s[l][fs, :].rearrange("(t p) f -> p t f", p=P))
                    b1_t = small.tile([P, KT], F32, tag="gain")
                    nc.sync.dma_start(b1_t, b1_s[l, g])
                    for qb in range(NQB):
                        qs = slice(qb * QB, (qb + 1) * QB)
                        ffT = vpool.tile([P, KT, QB], F32R, tag="fft")
                        for fc in range(KT):
                            ps = psA.tile([P, QB], F32, tag="psA")
                            for kt in range(KT):
                                nc.tensor.matmul(
                                    ps, w1_t[:, kt, fc * P:(fc + 1) * P],
                                    h2T[:, kt, qs], start=kt == 0, stop=kt == KT - 1)
                            nc.scalar.activation(
                                ffT[:, fc, :], ps, ACT.Gelu,
                                bias=b1_t[:, fc:fc + 1])
                        for oc in range(KT):
                            ps = psA.tile([P, QB], F32, tag="psA")
                            for kt in range(KT):
                                nc.tensor.matmul(
                                    ps, w2_t[:, kt, oc * P:(oc + 1) * P],
                                    ffT[:, kt, :], start=kt == 0, stop=kt == KT - 1)
                            accum(part2[:, oc, qs], ps, g == 0)
                add_residual(part2, b2_s[l])

            # ---- final LN + vocab-sharded head ----
            xfT = layer_norm(fng, fnb)
            for vc in range(NVC):
                vs = slice(vc * VC, (vc + 1) * VC)
                hw_t = wpool.tile([P, KT, VC], F32R, tag="w")
                nc.sync.dma_start(hw_t, hw_s[:, vs].rearrange("(t p) v -> p t v", p=P))
                hb_row = sums.tile([1, VC], F32, tag="rcp")
                nc.sync.dma_start(hb_row, hb_s[:, vs])
                hb_b = vpool.tile([P, VC], F32, tag="hbb")
                nc.gpsimd.partition_broadcast(hb_b, hb_row)
                for tc_ in range(NKT):
                    ps = psA.tile([P, QB], F32, tag="psA")
                    for kt in range(KT):
                        nc.tensor.matmul(
                            ps, xfT[:, kt, tc_ * P:(tc_ + 1) * P],
                            hw_t[:, kt, :], start=kt == 0, stop=kt == KT - 1)
                    lg = eppool.tile([P, VC], F32, tag="e")
                    nc.vector.tensor_tensor(lg, ps, hb_b, AL.add)
                    nc.sync.dma_start(logits[tc_ * P:(tc_ + 1) * P, vs], lg)

    nc.finalize()
    return nc


def _prep_inputs(inputs):
    f = np.ascontiguousarray
    tokens = np.asarray(inputs["tokens"])
    tok_emb = np.asarray(inputs["tok_emb"], np.float32)
    pos_emb = np.asarray(inputs["pos_emb"], np.float32)

    Lb = L_BODY

    def colmajor(a):  # [..., D] -> [..., P, KT] per-partition columns
        return f(a.reshape(*a.shape[:-1], KT, P).swapaxes(-1, -2).astype(np.float32))

    masks = (np.arange(P)[:, None, None] + P * np.arange(TPG)[None, :, None]
             <= np.arange(QB)[None, None, :]).astype(np.float32)

    b1 = np.asarray(inputs["b1"], np.float32)[:Lb]
    base = {
        "masks": masks,
        "wq_s": f(np.asarray(inputs["wq"], np.float32)[:Lb]),
        "wk_s": f(np.asarray(inputs["wk"], np.float32)[:Lb]),
        "wv_s": f(np.asarray(inputs["wv"], np.float32)[:Lb]),
        "wo_s": f(np.asarray(inputs["wo"], np.float32)[:Lb]),
        "w1_s": f(np.asarray(inputs["w1"], np.float32)[:Lb]),
        "w2_s": f(np.asarray(inputs["w2"], np.float32)[:Lb]),
        "ln1g": colmajor(np.asarray(inputs["ln1_g"], np.float32)[:Lb]),
        "ln1b": colmajor(np.asarray(inputs["ln1_b"], np.float32)[:Lb]),
        "ln2g": colmajor(np.asarray(inputs["ln2_g"], np.float32)[:Lb]),
        "ln2b": colmajor(np.asarray(inputs["ln2_b"], np.float32)[:Lb]),
        "bq_s": f(np.asarray(inputs["bq"], np.float32)[:Lb].reshape(Lb, H, DH).swapaxes(1, 2)),
        "bk_s": f(np.asarray(inputs["bk"], np.float32)[:Lb].reshape(Lb, H, DH).swapaxes(1, 2)),
        "bv_s": f(np.asarray(inputs["bv"], np.float32)[:Lb].reshape(Lb, TPG, DL)),
        "bo_s": colmajor(np.asarray(inputs["bo"], np.float32)[:Lb]),
        "b1_s": colmajor(b1.reshape(Lb, TPG, FFL)),
        "b2_s": colmajor(np.asarray(inputs["b2"], np.float32)[:Lb]),
        "fng": colmajor(np.asarray(inputs["fn_g"], np.float32)),
        "fnb": colmajor(np.asarray(inputs["fn_b"], np.float32)),
    }

    head_w = np.asarray(inputs["head_w"], np.float32)
    head_b = np.asarray(inputs["head_b"], np.float32)

    in_maps = []
    for c in range(NCORES):
        b = c // TPG
        g = c % TPG
        v0, vn = VSTART[g], VSLICE[g]
        hw_pad = np.zeros((D, VPAD), np.float32)
        hw_pad[:, :vn] = head_w[:, v0:v0 + vn]
        hb_pad = np.zeros((1, VPAD), np.float32)
        hb_pad[0, :vn] = head_b[v0:v0 + vn]
        x0 = tok_emb[tokens[b]] + pos_emb[:S]
        m = {"x0T": f(x0.T.astype(np.float32)), "hw_s": hw_pad, "hb_s": hb_pad}
        m.update(base)
        in_maps.append(m)
    return in_maps


def _get_nc():
    key = ("nc", L_BODY)
    if key not in _CACHE:
        _CACHE[key] = _build()
    return _CACHE[key]


def kernel(**inputs):
    nc = _get_nc()
    in_maps = _prep_inputs(inputs)
    res = bass_utils.run_bass_kernel_spmd(nc, in_maps, core_ids=list(range(NCORES)))
    out = np.empty((B, S, V), np.float32)
    for c in range(NCORES):
        b, g = c // TPG, c % TPG
        v0, vn = VSTART[g], VSLICE[g]
        out[b, :, v0:v0 + vn] = res.results[c]["logits"][:, :vn]
    return out



# revision 18
# speedup vs baseline: 1.7766x; 1.7766x over previous
"""GPT-2 small (L=12, D=768, H=12, S=1024, B=2, V=50257) forward pass on 8
Trainium2 NeuronCores via Bass/Tile.

Sharding: data-parallel over batch + vocab-parallel head, zero collectives
(cores 0-3 compute batch 0 redundantly, 4-7 batch 1; each core does 1/4 of
the vocab head for its batch).

This revision of the kernel targets the PE/DVE dual bottleneck of the
original (7.52 ms sim):
  - weights and matmul activations in bf16 (full-rate PE, half DMA/SBUF)
  - heads processed in pairs: Q/K projections at M=128, score matmuls
    row-packed two-per-issue via partition-base tile_position (K=64 each)
  - softmax denominator via a ones-column appended to the V stationary:
    the attn@V PSUM row 64 accumulates sum(exp) for free (kills the DVE
    ssum chain + partition_all_reduce)
  - all linear biases folded into K=1 matmul rows accumulated in PSUM
  - O-proj / FFN2 accumulate over weight slices in PSUM and add straight
    into the residual xT (no part/part2 staging in DVE)
  - LayerNorm sum / sum-of-squares via ones-column matmuls on PE
"""

import numpy as np
import ml_dtypes

import concourse.bass as bass
import concourse.tile as tile
from concourse import bacc, mybir
from concourse import bass_utils

F32 = mybir.dt.float32
F32R = mybir.dt.float32r
BF16 = mybir.dt.bfloat16
AL = mybir.AluOpType
ACT = mybir.ActivationFunctionType
BF_NP = ml_dtypes.bfloat16

# model dims
B, S, D, H, DH, F4, V, L = 2, 1024, 768, 12, 64, 3072, 50257, 12
P = 128
KT = D // P            # 6 k-tiles over the model dim
EPS = 1e-5
SCALE = 1.0 / np.sqrt(DH)

# sharding / tiling
NCORES = 8
TPG = 2                # weight column-slices per layer (g loop)
HPC = H // TPG         # heads per slice (6)
NPAIR = HPC // 2       # head pairs per slice (3)
DL = HPC * DH          # slice attn width 384
DLV = HPC * (DH + 1)   # V slice width with ones-columns 390
FFL = F4 // TPG        # slice ffn width 1536
NFC = FFL // P         # 12 fc tiles per g-slice
NF4T = F4 // P         # 24 k-tiles over ffn dim
QB = 512               # query block
NQB = S // QB
NKT = S // P           # key tiles
VC = 512               # vocab chunk
VPAD = 12800           # padded per-core vocab slice (25 chunks of 512)
NVC = VPAD // VC
VSLICE = [12565, 12564, 12564, 12564]
VSTART = [0, 12565, 25129, 37693]

L_BODY = L  # overridable before first kernel() call for debugging

_CACHE = {}


def _build():
    nc = bacc.Bacc("TRN2", target_bir_lowering=False, debug=False,
                   num_devices=NCORES)

    def di(name, shape, dt=F32):
        return nc.dram_tensor(name, shape, dt, kind="ExternalInput").ap()

    x0T = di("x0T", [D, S], F32R)
    masks = di("masks", [P, 896], BF16)
    onesc = di("onesc", [P, 2], F32R)        # col 0 f32r ones (LN sums lhsT)
    onescb = di("onescb", [P, 2], BF16)      # bf16 ones col (sq sums lhsT)
    onesr = di("onesr", [1, QB], BF16)       # ones row (bias-row moving op)
    wq_s = di("wq_s", [L_BODY, D, D], BF16)  # pre-scaled by SCALE
    wk_s = di("wk_s", [L_BODY, D, D], BF16)
    wv_s = di("wv_s", [L_BODY, D, TPG * DLV], BF16)  # ones-cols zeroed
    wo_s = di("wo_s", [L_BODY, D, D], BF16)
    w1_s = di("w1_s", [L_BODY, D, F4], BF16)
    w2_s = di("w2_s", [L_BODY, F4, D], BF16)
    ln1g = di("ln1g", [L_BODY, P, KT])
    ln1b = di("ln1b", [L_BODY, P, KT])
    ln2g = di("ln2g", [L_BODY, P, KT])
    ln2b = di("ln2b", [L_BODY, P, KT])
    bq_s = di("bq_s", [L_BODY, 1, D], BF16)  # pre-scaled by SCALE
    bk_s = di("bk_s", [L_BODY, 1, D], BF16)
    bv_s = di("bv_s", [L_BODY, 1, TPG * DLV], BF16)  # 1.0 at ones-cols
    bo_s = di("bo_s", [L_BODY, 1, D], BF16)
    b1_s = di("b1_s", [L_BODY, TPG, P, NFC])
    b2_s = di("b2_s", [L_BODY, 1, D], BF16)
    fng = di("fng", [P, KT])
    fnb = di("fnb", [P, KT])
    hw_s = di("hw_s", [D, VPAD], BF16)
    hb_s = di("hb_s", [1, VPAD])
    logits = nc.dram_tensor("logits", [S, VPAD], F32, kind="ExternalOutput").ap()

    from contextlib import ExitStack

    with tile.TileContext(nc) as tc:
        with ExitStack() as stack:
            stack.enter_context(nc.allow_low_precision(
                reason="intentional bf16 pipeline; validated vs reference"))
            pools = {}
            for pname, bufs, space in [
                    ("persist", 1, "SBUF"), ("slab", 1, "SBUF"),
                    ("wqk", 2, "SBUF"), ("wvp", 1, "SBUF"),
                    ("wop", 2, "SBUF"), ("w1p", 2, "SBUF"),
                    ("w2p", 2, "SBUF"), ("qk", 2, "SBUF"),
                    ("vp", 2, "SBUF"), ("op", 2, "SBUF"),
                    ("ff", 1, "SBUF"), ("sq", 2, "SBUF"),
                    ("ep", 4, "SBUF"), ("rows", 4, "SBUF"),
                    ("bc", 3, "SBUF"), ("tmp", 3, "SBUF"),
                    ("small", 2, "SBUF"),
                    ("psA", 3, "PSUM"), ("psQK", 2, "PSUM"),
                    ("psO", 3, "PSUM")]:
                pools[pname] = stack.enter_context(
                    tc.tile_pool(name=pname, bufs=bufs, space=space))
            persist, slab, wqk, wvp, wop, w1p, w2p = (
                pools["persist"], pools["slab"], pools["wqk"], pools["wvp"],
                pools["wop"], pools["w1p"], pools["w2p"])
            qkpool, vpool, opool, ffpool, sqpool, eppool = (
                pools["qk"], pools["vp"], pools["op"], pools["ff"],
                pools["sq"], pools["ep"])
            rows, bcpool, tmp, small = (
                pools["rows"], pools["bc"], pools["tmp"], pools["small"])
            psA, psQK, psO = pools["psA"], pools["psQK"], pools["psO"]

            xTs = []
            for kt in range(KT):
                xkt = persist.tile([P, S], F32R, tag=f"xT{kt}")
                nc.sync.dma_start(xkt, x0T[kt * P:(kt + 1) * P, :])
                xTs.append(xkt)
            masks_sb = persist.tile([P, 896], BF16)
            nc.sync.dma_start(masks_sb, masks)
            ones_c = persist.tile([P, 2], F32R)
            nc.sync.dma_start(ones_c, onesc)
            ones_cb = persist.tile([P, 2], BF16)
            nc.sync.dma_start(ones_cb, onescb)
            ones_r = persist.tile([1, QB], BF16)
            nc.sync.dma_start(ones_r, onesr)

            def layer_norm(g_ap, b_ap):
                """LN over the feature axis of xT (128 partitions x KT),
                done per query block; writes a bf16 slab tile."""
                g_t = small.tile([P, KT], F32, tag="gain")
                b_t = small.tile([P, KT], F32, tag="gain")
                nc.sync.dma_start(g_t, g_ap)
                nc.sync.dma_start(b_t, b_ap)
                out = slab.tile([P, KT, S], BF16, tag="slab")
                for qb in range(NQB):
                    qs = slice(qb * QB, (qb + 1) * QB)
                    # sum and sum-of-squares via ones-column matmuls
                    acc = psO.tile([DH + 1, QB], F32, tag="ps65")
                    accsq = psO.tile([DH + 1, QB], F32, tag="ps65")
                    for kt in range(KT):
                        nc.tensor.matmul(
                            acc[0:1, :], ones_c[:, 0:1], xTs[kt][:, qs],
                            start=kt == 0, stop=kt == KT - 1)
                    for kt in range(KT):
                        sq = sqpool.tile([P, QB], BF16, tag="sq")
                        nc.scalar.activation(sq, xTs[kt][:, qs], ACT.Square)
                        nc.tensor.matmul(
                            accsq[0:1, :], ones_cb[:, 0:1], sq,
                            start=kt == 0, stop=kt == KT - 1)
                    m_row = rows.tile([1, QB], F32, tag="row")
                    r_row = rows.tile([1, QB], F32, tag="row")
                    t_row = rows.tile([1, QB], F32, tag="row")
                    nc.vector.tensor_scalar_mul(m_row, acc[0:1, :], 1.0 / D)
                    nc.vector.tensor_scalar_mul(r_row, accsq[0:1, :], 1.0 / D)
                    nc.vector.tensor_tensor(t_row, m_row, m_row, AL.mult)
                    nc.vector.tensor_tensor(r_row, r_row, t_row, AL.subtract)
                    nc.vector.tensor_scalar_add(r_row, r_row, EPS)
                    nc.scalar.activation(r_row, r_row, ACT.Sqrt)
                    nc.vector.reciprocal(r_row, r_row)
                    m_b = bcpool.tile([P, QB], F32, tag="bc")
                    r_b = bcpool.tile([P, QB], F32, tag="bc")
                    nc.gpsimd.partition_broadcast(m_b, m_row)
                    nc.gpsimd.partition_broadcast(r_b, r_row)
                    for kt in range(KT):
                        t = tmp.tile([P, QB], F32, tag="acc")
                        nc.vector.tensor_tensor(t, xTs[kt][:, qs], m_b,
                                                AL.subtract)
                        nc.vector.tensor_tensor(t, t, r_b, AL.mult)
                        nc.vector.tensor_scalar(
                            out[:, kt, qs], t, g_t[:, kt:kt + 1],
                            b_t[:, kt:kt + 1], AL.mult, AL.add)
                return out

            for l in range(L_BODY):
                # ---- attention ----
                hT = layer_norm(ln1g[l], ln1b[l])
                oTs = []
                wos = []
                for g in range(TPG):
                    cs = slice(g * DL, (g + 1) * DL)
                    csv = slice(g * DLV, (g + 1) * DLV)
                    wv_t = wvp.tile([P, KT, DLV], BF16, tag="wv")
                    nc.sync.dma_start(
                        wv_t, wv_s[l][:, csv].rearrange("(t p) f -> p t f", p=P))
                    wo_t = wop.tile([P, NPAIR, D], BF16, tag="wo")
                    r0 = g * DL
                    nc.sync.dma_start(
                        wo_t, wo_s[l][r0:r0 + DL, :].rearrange(
                            "(t p) f -> p t f", p=P))
                    wos.append(wo_t)
                    brow = small.tile([1, 2 * DL + DLV], BF16, tag="brow")
                    nc.sync.dma_start(brow[:, 0:DL], bq_s[l][:, cs])
                    nc.sync.dma_start(brow[:, DL:2 * DL], bk_s[l][:, cs])
                    nc.sync.dma_start(brow[:, 2 * DL:], bv_s[l][:, csv])

                    qT = qkpool.tile([P, NPAIR, S], F32R, tag="qk")
                    kTt = qkpool.tile([P, NPAIR, S], F32R, tag="qk")
                    for p in range(NPAIR):
                        gc = g * DL + p * P
                        wq_t = wqk.tile([P, KT, P], BF16, tag="wqk")
                        nc.sync.dma_start(
                            wq_t, wq_s[l][:, gc:gc + P].rearrange(
                                "(t p) f -> p t f", p=P))
                        wk_t = wqk.tile([P, KT, P], BF16, tag="wqk")
                        nc.sync.dma_start(
                            wk_t, wk_s[l][:, gc:gc + P].rearrange(
                                "(t p) f -> p t f", p=P))
                        for qb in range(NQB):
                            qs = slice(qb * QB, (qb + 1) * QB)
                            q_ps = psQK.tile([P, QB], F32, tag="psQK")
                            for kt in range(KT):
                                nc.tensor.matmul(
                                    q_ps, wq_t[:, kt, :], hT[:, kt, qs],
                                    start=kt == 0, stop=False)
                            nc.tensor.matmul(
                                q_ps, brow[:, p * P:p * P + P],
                                ones_r, start=False, stop=True)
                            nc.vector.tensor_copy(qT[:, p, qs], q_ps)
                            k_ps = psQK.tile([P, QB], F32, tag="psQK")
                            for kt in range(KT):
                                nc.tensor.matmul(
                                    k_ps, wk_t[:, kt, :], hT[:, kt, qs],
                                    start=kt == 0, stop=False)
                            nc.tensor.matmul(
                                k_ps,
                                brow[:, DL + p * P:DL + p * P + P],
                                ones_r, start=False, stop=True)
                            nc.vector.tensor_copy(kTt[:, p, qs], k_ps)

                    v_t = vpool.tile([P, NKT, DLV], BF16, tag="v")
                    for tc_ in range(NKT):
                        v_ps = psA.tile([P, QB], F32, tag="psA")
                        for kt in range(KT):
                            nc.tensor.matmul(
                                v_ps[:, :DLV], hT[:, kt, tc_ * P:(tc_ + 1) * P],
                                wv_t[:, kt, :], start=kt == 0, stop=False)
                        nc.tensor.matmul(
                            v_ps[:, :DLV], ones_r[:, 0:P],
                            brow[:, 2 * DL:2 * DL + DLV], start=False, stop=True)
                        nc.vector.tensor_copy(v_t[:, tc_, :], v_ps[:, :DLV])

                    oT = opool.tile([P, NPAIR, S], BF16, tag="o")
                    oTs.append(oT)
                    for p in range(NPAIR):
                        for qb in range(NQB):
                            qs = slice(qb * QB, (qb + 1) * QB)
                            nkt = 4 * qb + 4
                            oA = psO.tile([DH + 1, QB], F32, tag="ps65")
                            oB = psO.tile([DH + 1, QB], F32, tag="ps65")
                            for ti in range(nkt):
                                ks = slice(ti * P, (ti + 1) * P)
                                sA = psA.tile([P, QB], F32, tag="psA")
                                sB = psQK.tile([P, QB], F32, tag="psQK")
                                nc.tensor.matmul(sA, kTt[0:DH, p, ks],
                                                 qT[0:DH, p, qs],
                                                 start=True, stop=True)
                                nc.tensor.matmul(sB, kTt[DH:P, p, ks],
                                                 qT[DH:P, p, qs],
                                                 start=True, stop=True)
                                eA = eppool.tile([P, QB], BF16, tag="e")
                                eB = eppool.tile([P, QB], BF16, tag="e")
                                nc.scalar.activation(eA, sA, ACT.Exp)
                                nc.scalar.activation(eB, sB, ACT.Exp)
                                r = ti - 4 * qb
                                if r >= 0:
                                    ms = masks_sb[:, 384 - 128 * r:
                                                  384 - 128 * r + QB]
                                    nc.vector.tensor_tensor(eA, eA, ms, AL.mult)
                                    nc.vector.tensor_tensor(eB, eB, ms, AL.mult)
                                va = (2 * p) * (DH + 1)
                                vb = (2 * p + 1) * (DH + 1)
                                nc.tensor.matmul(
                                    oA, v_t[:, ti, va:va + DH + 1], eA,
                                    start=ti == 0, stop=ti == nkt - 1)
                                nc.tensor.matmul(
                                    oB, v_t[:, ti, vb:vb + DH + 1], eB,
                                    start=ti == 0, stop=ti == nkt - 1)
                            for o_ps, half in ((oA, 0), (oB, 1)):
                                rcp = rows.tile([1, QB], F32, tag="row")
                                nc.vector.reciprocal(rcp, o_ps[DH:DH + 1, :])
                                rcp_b = bcpool.tile([DH, QB], F32, tag="bc")
                                nc.gpsimd.partition_broadcast(rcp_b, rcp)
                                nc.vector.tensor_tensor(
                                    oT[half * DH:half * DH + DH, p, qs],
                                    o_ps[0:DH, :], rcp_b, AL.mult)

                # O-projection: accumulate both g-slices in PSUM, add into xT
                borow = small.tile([1, D], BF16, tag="borow")
                nc.sync.dma_start(borow, bo_s[l])
                for oc in range(KT):
                    ocs = slice(oc * P, (oc + 1) * P)
                    for qb in range(NQB):
                        qs = slice(qb * QB, (qb + 1) * QB)
                        ps = psA.tile([P, QB], F32, tag="psA")
                        for g in range(TPG):
                            for r in range(NPAIR):
                                nc.tensor.matmul(
                                    ps, wos[g][:, r, ocs], oTs[g][:, r, qs],
                                    start=(g == 0 and r == 0), stop=False)
                        nc.tensor.matmul(ps, borow[:, ocs], ones_r,
                                         start=False, stop=True)
                        nc.vector.tensor_tensor(xTs[oc][:, qs], xTs[oc][:, qs],
                                                ps, AL.add)

                # ---- ffn ----
                h2T = layer_norm(ln2g[l], ln2b[l])
                b2row = small.tile([1, D], BF16, tag="borow")
                nc.sync.dma_start(b2row, b2_s[l])
                for qb in range(NQB):
                    qs = slice(qb * QB, (qb + 1) * QB)
                    ffT = ffpool.tile([P, NF4T, QB], BF16, tag="fft")
                    for g in range(TPG):
                        b1_t = small.tile([P, NFC], F32, tag="b1")
                        nc.sync.dma_start(b1_t, b1_s[l, g])
                        for fh in range(4):
                            fs = slice(g * FFL + fh * (FFL // 4),
                                       g * FFL + (fh + 1) * (FFL // 4))
                            w1_t = w1p.tile([P, KT, FFL // 4], BF16, tag="w1")
                            nc.sync.dma_start(
                                w1_t, w1_s[l][:, fs].rearrange(
                                    "(t p) f -> p t f", p=P))
                            for fc in range(NFC // 4):
                                ps = psA.tile([P, QB], F32, tag="psA")
                                for kt in range(KT):
                                    nc.tensor.matmul(
                                        ps, w1_t[:, kt, fc * P:(fc + 1) * P],
                                        h2T[:, kt, qs],
                                        start=kt == 0, stop=kt == KT - 1)
                                fl = fh * (NFC // 4) + fc
                                fidx = g * NFC + fl
                                nc.scalar.activation(
                                    ffT[:, fidx, :], ps, ACT.Gelu,
                                    bias=b1_t[:, fl:fl + 1])
                    for oc in range(KT):
                        ocs = slice(oc * P, (oc + 1) * P)
                        w2_t = w2p.tile([P, NF4T, P], BF16, tag="w2")
                        nc.sync.dma_start(
                            w2_t, w2_s[l][:, ocs].rearrange(
                                "(t p) f -> p t f", p=P))
                        ps = psA.tile([P, QB], F32, tag="psA")
                        for f4t in range(NF4T):
                            nc.tensor.matmul(
                                ps, w2_t[:, f4t, :], ffT[:, f4t, :],
                                start=f4t == 0, stop=False)
                        nc.tensor.matmul(ps, b2row[:, ocs], ones_r,
                                         start=False, stop=True)
                        nc.vector.tensor_tensor(xTs[oc][:, qs], xTs[oc][:, qs],
                                                ps, AL.add)

            # ---- final LN + vocab-sharded head ----
            xfT = layer_norm(fng, fnb)
            for vc in range(NVC):
                vs = slice(vc * VC, (vc + 1) * VC)
                hw_t = qkpool.tile([P, KT, VC], BF16, tag="qk")
                nc.sync.dma_start(hw_t, hw_s[:, vs].rearrange("(t p) v -> p t v", p=P))
                hb_rf = rows.tile([1, VC], F32, tag="row")
                nc.sync.dma_start(hb_rf, hb_s[:, vs])
                hb_row = rows.tile([1, VC], F32, tag="row")
                nc.vector.tensor_copy(hb_row, hb_rf)
                hb_b = bcpool.tile([P, VC], F32, tag="bc")
                nc.gpsimd.partition_broadcast(hb_b, hb_row)
                for tc_ in range(NKT):
                    ps = psA.tile([P, QB], F32, tag="psA")
                    for kt in range(KT):
                        nc.tensor.matmul(
                            ps, xfT[:, kt, tc_ * P:(tc_ + 1) * P],
                            hw_t[:, kt, :], start=kt == 0, stop=kt == KT - 1)
                    lg = tmp.tile([P, VC], F32, tag="acc")
                    nc.vector.tensor_tensor(lg, ps, hb_b, AL.add)
                    nc.sync.dma_start(logits[tc_ * P:(tc_ + 1) * P, vs], lg)

    nc.finalize()
    return nc


def _prep_inputs(inputs):
    f = np.ascontiguousarray
    tokens = np.asarray(inputs["tokens"])
    tok_emb = np.asarray(inputs["tok_emb"], np.float32)
    pos_emb = np.asarray(inputs["pos_emb"], np.float32)

    Lb = L_BODY

    def bf(a):
        return f(np.asarray(a, np.float32).astype(BF_NP))

    def colmajor(a):  # [..., D] -> [..., P, KT] per-partition columns
        return f(a.reshape(*a.shape[:-1], KT, P).swapaxes(-1, -2).astype(np.float32))

    # multiplicative causal masks for the diagonal key tiles, bf16
    masks = (np.arange(P)[:, None] + 384
             <= np.arange(896)[None, :]).astype(BF_NP)

    wv = np.asarray(inputs["wv"], np.float32)[:Lb]          # [L, D, D]
    bv = np.asarray(inputs["bv"], np.float32)[:Lb]          # [L, D]
    # insert ones-columns: per head h, cols h*65..h*65+63 = V, col h*65+64 pad
    wv_p = np.zeros((Lb, D, H * (DH + 1)), np.float32)
    bv_p = np.zeros((Lb, 1, H * (DH + 1)), np.float32)
    for h in range(H):
        wv_p[:, :, h * (DH + 1):h * (DH + 1) + DH] = wv[:, :, h * DH:(h + 1) * DH]
        bv_p[:, 0, h * (DH + 1):h * (DH + 1) + DH] = bv[:, h * DH:(h + 1) * DH]
        bv_p[:, 0, h * (DH + 1) + DH] = 1.0

    b1 = np.asarray(inputs["b1"], np.float32)[:Lb]

    base = {
        "masks": masks,
        "onesc": np.ones((P, 2), np.float32),
        "onescb": np.ones((P, 2), BF_NP),
        "onesr": np.ones((1, QB), BF_NP),
        "wq_s": bf(np.asarray(inputs["wq"], np.float32)[:Lb] * SCALE),
        "wk_s": bf(np.asarray(inputs["wk"], np.float32)[:Lb]),
        "wv_s": bf(wv_p),
        "wo_s": bf(np.asarray(inputs["wo"], np.float32)[:Lb]),
        "w1_s": bf(np.asarray(inputs["w1"], np.float32)[:Lb]),
        "w2_s": bf(np.asarray(inputs["w2"], np.float32)[:Lb]),
        "ln1g": colmajor(np.asarray(inputs["ln1_g"], np.float32)[:Lb]),
        "ln1b": colmajor(np.asarray(inputs["ln1_b"], np.float32)[:Lb]),
        "ln2g": colmajor(np.asarray(inputs["ln2_g"], np.float32)[:Lb]),
        "ln2b": colmajor(np.asarray(inputs["ln2_b"], np.float32)[:Lb]),
        "bq_s": bf((np.asarray(inputs["bq"], np.float32)[:Lb] * SCALE)[:, None, :]),
        "bk_s": bf(np.asarray(inputs["bk"], np.float32)[:Lb][:, None, :]),
        "bv_s": bf(bv_p),
        "bo_s": bf(np.asarray(inputs["bo"], np.float32)[:Lb][:, None, :]),
        "b1_s": f(b1.reshape(Lb, TPG, NFC, P).swapaxes(-1, -2).astype(np.float32)),
        "b2_s": bf(np.asarray(inputs["b2"], np.float32)[:Lb][:, None, :]),
        "fng": colmajor(np.asarray(inputs["fn_g"], np.float32)),
        "fnb": colmajor(np.asarray(inputs["fn_b"], np.float32)),
    }

    head_w = np.asarray(inputs["head_w"], np.float32)
    head_b = np.asarray(inputs["head_b"], np.float32)

    in_maps = []
    for c in range(NCORES):
        b = c // 4
        gsl = c % 4
        v0, vn = VSTART[gsl], VSLICE[gsl]
        hw_pad = np.zeros((D, VPAD), BF_NP)
        hw_pad[:, :vn] = head_w[:, v0:v0 + vn].astype(BF_NP)
        hb_pad = np.zeros((1, VPAD), np.float32)
        hb_pad[0, :vn] = head_b[v0:v0 + vn]
        x0 = tok_emb[tokens[b]] + pos_emb[:S]
        m = {"x0T": f(x0.T.astype(np.float32)), "hw_s": hw_pad, "hb_s": hb_pad}
        m.update(base)
        in_maps.append(m)
    return in_maps


def _get_nc():
    key = ("nc", L_BODY)
    if key not in _CACHE:
        _CACHE[key] = _build()
    return _CACHE[key]


def kernel(**inputs):
    nc = _get_nc()
    in_maps = _prep_inputs(inputs)
    res = bass_utils.run_bass_kernel_spmd(nc, in_maps, core_ids=list(range(NCORES)))
    out = np.empty((B, S, V), np.float32)
    for c in range(NCORES):
        b, gsl = c // 4, c % 4
        v0, vn = VSTART[gsl], VSLICE[gsl]
        out[b, :, v0:v0 + vn] = res.results[c]["logits"][:, :vn]
    return out


# revision 21
# speedup vs baseline: 1.8015x; 1.0140x over previous
"""GPT-2 small (L=12, D=768, H=12, S=1024, B=2, V=50257) forward pass on 8
Trainium2 NeuronCores via Bass/Tile.

Sharding: data-parallel over batch + vocab-parallel head, zero collectives
(cores 0-3 compute batch 0 redundantly, 4-7 batch 1; each core does 1/4 of
the vocab head for its batch).

This revision of the kernel targets the PE/DVE dual bottleneck of the
original (7.52 ms sim):
  - weights and matmul activations in bf16 (full-rate PE, half DMA/SBUF)
  - heads processed in pairs: Q/K projections at M=128, score matmuls
    row-packed two-per-issue via partition-base tile_position (K=64 each)
  - softmax denominator via a ones-column appended to the V stationary:
    the attn@V PSUM row 64 accumulates sum(exp) for free (kills the DVE
    ssum chain + partition_all_reduce)
  - all linear biases folded into K=1 matmul rows accumulated in PSUM
  - O-proj / FFN2 accumulate over weight slices in PSUM and add straight
    into the residual xT (no part/part2 staging in DVE)
  - LayerNorm sum / sum-of-squares via ones-column matmuls on PE
"""

import numpy as np
import ml_dtypes

import concourse.bass as bass
import concourse.tile as tile
from concourse import bacc, mybir
from concourse import bass_utils

F32 = mybir.dt.float32
F32R = mybir.dt.float32r
BF16 = mybir.dt.bfloat16
AL = mybir.AluOpType
ACT = mybir.ActivationFunctionType
BF_NP = ml_dtypes.bfloat16

# model dims
B, S, D, H, DH, F4, V, L = 2, 1024, 768, 12, 64, 3072, 50257, 12
P = 128
KT = D // P            # 6 k-tiles over the model dim
EPS = 1e-5
SCALE = 1.0 / np.sqrt(DH)

# sharding / tiling
NCORES = 8
TPG = 2                # weight column-slices per layer (g loop)
HPC = H // TPG         # heads per slice (6)
NPAIR = HPC // 2       # head pairs per slice (3)
DL = HPC * DH          # slice attn width 384
DLV = HPC * (DH + 1)   # V slice width with ones-columns 390
FFL = F4 // TPG        # slice ffn width 1536
NFC = FFL // P         # 12 fc tiles per g-slice
NF4T = F4 // P         # 24 k-tiles over ffn dim
QB = 512               # query block
NQB = S // QB
NKT = S // P           # key tiles
VC = 512               # vocab chunk
VPAD = 12800           # padded per-core vocab slice (25 chunks of 512)
NVC = VPAD // VC
VSLICE = [12565, 12564, 12564, 12564]
VSTART = [0, 12565, 25129, 37693]

L_BODY = L  # overridable before first kernel() call for debugging

_CACHE = {}


def _build():
    nc = bacc.Bacc("TRN2", target_bir_lowering=False, debug=False,
                   num_devices=NCORES)

    def di(name, shape, dt=F32):
        return nc.dram_tensor(name, shape, dt, kind="ExternalInput").ap()

    x0T = di("x0T", [D, S], F32R)
    masks = di("masks", [P, 896], BF16)
    onesc = di("onesc", [P, 2], F32R)        # col 0 f32r ones (LN sums lhsT)
    onescb = di("onescb", [P, 2], BF16)      # bf16 ones col (sq sums lhsT)
    onesr = di("onesr", [1, QB], BF16)       # ones row (bias-row moving op)
    wq_s = di("wq_s", [L_BODY, D, D], BF16)  # pre-scaled by SCALE
    wk_s = di("wk_s", [L_BODY, D, D], BF16)
    wv_s = di("wv_s", [L_BODY, D, TPG * DLV], BF16)  # ones-cols zeroed
    wo_s = di("wo_s", [L_BODY, D, D], BF16)
    w1_s = di("w1_s", [L_BODY, D, F4], BF16)
    w2_s = di("w2_s", [L_BODY, F4, D], BF16)
    ln1g = di("ln1g", [L_BODY, P, KT])
    ln1b = di("ln1b", [L_BODY, P, KT])
    ln2g = di("ln2g", [L_BODY, P, KT])
    ln2b = di("ln2b", [L_BODY, P, KT])
    bq_s = di("bq_s", [L_BODY, 1, D], BF16)  # pre-scaled by SCALE
    bk_s = di("bk_s", [L_BODY, 1, D], BF16)
    bv_s = di("bv_s", [L_BODY, 1, TPG * DLV], BF16)  # 1.0 at ones-cols
    bo_s = di("bo_s", [L_BODY, 1, D], BF16)
    b1_s = di("b1_s", [L_BODY, TPG, P, NFC])
    b2_s = di("b2_s", [L_BODY, 1, D], BF16)
    fng = di("fng", [P, KT])
    fnb = di("fnb", [P, KT])
    hw_s = di("hw_s", [D, VPAD], BF16)
    hb_s = di("hb_s", [1, VPAD])
    logits = nc.dram_tensor("logits", [S, VPAD], F32, kind="ExternalOutput").ap()

    from contextlib import ExitStack

    with tile.TileContext(nc) as tc:
        with ExitStack() as stack:
            stack.enter_context(nc.allow_low_precision(
                reason="intentional bf16 pipeline; validated vs reference"))
            pools = {}
            for pname, bufs, space in [
                    ("persist", 1, "SBUF"), ("slab", 1, "SBUF"),
                    ("wqk", 2, "SBUF"), ("wvp", 1, "SBUF"),
                    ("wop", 2, "SBUF"), ("w1p", 2, "SBUF"),
                    ("w2p", 2, "SBUF"), ("qk", 2, "SBUF"),
                    ("vp", 2, "SBUF"), ("op", 2, "SBUF"),
                    ("ff", 1, "SBUF"), ("sq", 2, "SBUF"),
                    ("ep", 6, "SBUF"), ("rows", 4, "SBUF"),
                    ("bc", 3, "SBUF"), ("tmp", 3, "SBUF"),
                    ("small", 2, "SBUF"),
                    ("psA", 3, "PSUM"), ("psQK", 2, "PSUM"),
                    ("psO", 3, "PSUM")]:
                pools[pname] = stack.enter_context(
                    tc.tile_pool(name=pname, bufs=bufs, space=space))
            persist, slab, wqk, wvp, wop, w1p, w2p = (
                pools["persist"], pools["slab"], pools["wqk"], pools["wvp"],
                pools["wop"], pools["w1p"], pools["w2p"])
            qkpool, vpool, opool, ffpool, sqpool, eppool = (
                pools["qk"], pools["vp"], pools["op"], pools["ff"],
                pools["sq"], pools["ep"])
            rows, bcpool, tmp, small = (
                pools["rows"], pools["bc"], pools["tmp"], pools["small"])
            psA, psQK, psO = pools["psA"], pools["psQK"], pools["psO"]

            xTs = []
            for kt in range(KT):
                row = []
                for qb in range(NQB):
                    xkt = persist.tile([P, QB], F32R, tag=f"xT{kt}_{qb}")
                    nc.sync.dma_start(
                        xkt, x0T[kt * P:(kt + 1) * P,
                                 qb * QB:(qb + 1) * QB])
                    row.append(xkt)
                xTs.append(row)
            masks_sb = persist.tile([P, 896], BF16)
            nc.sync.dma_start(masks_sb, masks)
            ones_c = persist.tile([P, 2], F32R)
            nc.sync.dma_start(ones_c, onesc)
            ones_cb = persist.tile([P, 2], BF16)
            nc.sync.dma_start(ones_cb, onescb)
            ones_r = persist.tile([1, QB], BF16)
            nc.sync.dma_start(ones_r, onesr)

            def layer_norm(g_ap, b_ap):
                """LN over the feature axis of xT (128 partitions x KT),
                done per query block; writes a bf16 slab tile."""
                g_t = small.tile([P, KT], F32, tag="gain")
                b_t = small.tile([P, KT], F32, tag="gain")
                nc.sync.dma_start(g_t, g_ap)
                nc.sync.dma_start(b_t, b_ap)
                out = slab.tile([P, KT, S], BF16, tag="slab")
                for qb in range(NQB):
                    qs = slice(qb * QB, (qb + 1) * QB)
                    # sum and sum-of-squares via ones-column matmuls
                    acc = psO.tile([DH + 1, QB], F32, tag="ps65")
                    accsq = psO.tile([DH + 1, QB], F32, tag="ps65")
                    for kt in range(KT):
                        nc.tensor.matmul(
                            acc[0:1, :], ones_c[:, 0:1], xTs[kt][qb],
                            start=kt == 0, stop=kt == KT - 1)
                    for kt in range(KT):
                        sq = sqpool.tile([P, QB], BF16, tag="sq")
                        nc.scalar.activation(sq, xTs[kt][qb], ACT.Square)
                        nc.tensor.matmul(
                            accsq[0:1, :], ones_cb[:, 0:1], sq,
                            start=kt == 0, stop=kt == KT - 1)
                    m_row = rows.tile([1, QB], F32, tag="row")
                    r_row = rows.tile([1, QB], F32, tag="row")
                    t_row = rows.tile([1, QB], F32, tag="row")
                    nc.vector.tensor_scalar_mul(m_row, acc[0:1, :], 1.0 / D)
                    nc.vector.tensor_scalar_mul(r_row, accsq[0:1, :], 1.0 / D)
                    nc.vector.tensor_tensor(t_row, m_row, m_row, AL.mult)
                    nc.vector.tensor_tensor(r_row, r_row, t_row, AL.subtract)
                    nc.vector.tensor_scalar_add(r_row, r_row, EPS)
                    nc.scalar.activation(r_row, r_row, ACT.Sqrt)
                    nc.vector.reciprocal(r_row, r_row)
                    m_b = bcpool.tile([P, QB], F32, tag="bc")
                    r_b = bcpool.tile([P, QB], F32, tag="bc")
                    nc.gpsimd.partition_broadcast(m_b, m_row)
                    nc.gpsimd.partition_broadcast(r_b, r_row)
                    for kt in range(KT):
                        t = tmp.tile([P, QB], F32, tag="acc")
                        nc.vector.tensor_tensor(t, xTs[kt][qb], m_b,
                                                AL.subtract)
                        nc.vector.tensor_tensor(t, t, r_b, AL.mult)
                        nc.vector.tensor_scalar(
                            out[:, kt, qs], t, g_t[:, kt:kt + 1],
                            b_t[:, kt:kt + 1], AL.mult, AL.add)
                return out

            for l in range(L_BODY):
                # ---- attention ----
                hT = layer_norm(ln1g[l], ln1b[l])
                oTs = []
                wos = []
                for g in range(TPG):
                    cs = slice(g * DL, (g + 1) * DL)
                    csv = slice(g * DLV, (g + 1) * DLV)
                    wv_t = wvp.tile([P, KT, DLV], BF16, tag="wv")
                    nc.sync.dma_start(
                        wv_t, wv_s[l][:, csv].rearrange("(t p) f -> p t f", p=P))
                    wo_t = wop.tile([P, NPAIR, D], BF16, tag="wo")
                    r0 = g * DL
                    nc.sync.dma_start(
                        wo_t, wo_s[l][r0:r0 + DL, :].rearrange(
                            "(t p) f -> p t f", p=P))
                    wos.append(wo_t)
                    brow = small.tile([1, 2 * DL + DLV], BF16, tag="brow")
                    nc.sync.dma_start(brow[:, 0:DL], bq_s[l][:, cs])
                    nc.sync.dma_start(brow[:, DL:2 * DL], bk_s[l][:, cs])
                    nc.sync.dma_start(brow[:, 2 * DL:], bv_s[l][:, csv])

                    qT = qkpool.tile([P, NPAIR, S], F32R, tag="qk")
                    kTt = qkpool.tile([P, NPAIR, S], F32R, tag="qk")
                    for p in range(NPAIR):
                        gc = g * DL + p * P
                        wq_t = wqk.tile([P, KT, P], BF16, tag="wqk")
                        nc.sync.dma_start(
                            wq_t, wq_s[l][:, gc:gc + P].rearrange(
                                "(t p) f -> p t f", p=P))
                        wk_t = wqk.tile([P, KT, P], BF16, tag="wqk")
                        nc.sync.dma_start(
                            wk_t, wk_s[l][:, gc:gc + P].rearrange(
                                "(t p) f -> p t f", p=P))
                        for qb in range(NQB):
                            qs = slice(qb * QB, (qb + 1) * QB)
                            q_ps = psQK.tile([P, QB], F32, tag="psQK")
                            for kt in range(KT):
                                nc.tensor.matmul(
                                    q_ps, wq_t[:, kt, :], hT[:, kt, qs],
                                    start=kt == 0, stop=False)
                            nc.tensor.matmul(
                                q_ps, brow[:, p * P:p * P + P],
                                ones_r, start=False, stop=True)
                            nc.vector.tensor_copy(qT[:, p, qs], q_ps)
                            k_ps = psQK.tile([P, QB], F32, tag="psQK")
                            for kt in range(KT):
                                nc.tensor.matmul(
                                    k_ps, wk_t[:, kt, :], hT[:, kt, qs],
                                    start=kt == 0, stop=False)
                            nc.tensor.matmul(
                                k_ps,
                                brow[:, DL + p * P:DL + p * P + P],
                                ones_r, start=False, stop=True)
                            nc.vector.tensor_copy(kTt[:, p, qs], k_ps)

                    v_t = vpool.tile([P, NKT, DLV], BF16, tag="v")
                    for tc_ in range(NKT):
                        v_ps = psA.tile([P, QB], F32, tag="psA")
                        for kt in range(KT):
                            nc.tensor.matmul(
                                v_ps[:, :DLV], hT[:, kt, tc_ * P:(tc_ + 1) * P],
                                wv_t[:, kt, :], start=kt == 0, stop=False)
                        nc.tensor.matmul(
                            v_ps[:, :DLV], ones_r[:, 0:P],
                            brow[:, 2 * DL:2 * DL + DLV], start=False, stop=True)
                        nc.vector.tensor_copy(v_t[:, tc_, :], v_ps[:, :DLV])

                    oT = opool.tile([P, NPAIR, S], BF16, tag="o")
                    oTs.append(oT)
                    for p in range(NPAIR):
                        for qb in range(NQB):
                            qs = slice(qb * QB, (qb + 1) * QB)
                            nkt = 4 * qb + 4
                            oA = psO.tile([DH + 1, QB], F32, tag="ps65")
                            oB = psO.tile([DH + 1, QB], F32, tag="ps65")
                            for ti in range(nkt):
                                ks = slice(ti * P, (ti + 1) * P)
                                sA = psA.tile([P, QB], F32, tag="psA")
                                sB = psQK.tile([P, QB], F32, tag="psQK")
                                nc.tensor.matmul(sA, kTt[0:DH, p, ks],
                                                 qT[0:DH, p, qs],
                                                 start=True, stop=True)
                                nc.tensor.matmul(sB, kTt[DH:P, p, ks],
                                                 qT[DH:P, p, qs],
                                                 start=True, stop=True)
                                eA = eppool.tile([P, QB], BF16, tag="e")
                                eB = eppool.tile([P, QB], BF16, tag="e")
                                nc.scalar.activation(eA, sA, ACT.Exp)
                                nc.scalar.activation(eB, sB, ACT.Exp)
                                r = ti - 4 * qb
                                if r >= 0:
                                    ms = masks_sb[:, 384 - 128 * r:
                                                  384 - 128 * r + QB]
                                    nc.vector.tensor_tensor(eA, eA, ms, AL.mult)
                                    nc.vector.tensor_tensor(eB, eB, ms, AL.mult)
                                va = (2 * p) * (DH + 1)
                                vb = (2 * p + 1) * (DH + 1)
                                nc.tensor.matmul(
                                    oA, v_t[:, ti, va:va + DH + 1], eA,
                                    start=ti == 0, stop=ti == nkt - 1)
                                nc.tensor.matmul(
                                    oB, v_t[:, ti, vb:vb + DH + 1], eB,
                                    start=ti == 0, stop=ti == nkt - 1)
                            for o_ps, half in ((oA, 0), (oB, 1)):
                                rcp = rows.tile([1, QB], F32, tag="row")
                                nc.vector.reciprocal(rcp, o_ps[DH:DH + 1, :])
                                rcp_b = bcpool.tile([DH, QB], F32, tag="bc")
                                nc.gpsimd.partition_broadcast(rcp_b, rcp)
                                nc.vector.tensor_tensor(
                                    oT[half * DH:half * DH + DH, p, qs],
                                    o_ps[0:DH, :], rcp_b, AL.mult)

                # O-projection: accumulate both g-slices in PSUM, add into xT
                borow = small.tile([1, D], BF16, tag="borow")
                nc.sync.dma_start(borow, bo_s[l])
                for oc in range(KT):
                    ocs = slice(oc * P, (oc + 1) * P)
                    for qb in range(NQB):
                        qs = slice(qb * QB, (qb + 1) * QB)
                        ps = psA.tile([P, QB], F32, tag="psA")
                        for g in range(TPG):
                            for r in range(NPAIR):
                                nc.tensor.matmul(
                                    ps, wos[g][:, r, ocs], oTs[g][:, r, qs],
                                    start=(g == 0 and r == 0), stop=False)
                        nc.tensor.matmul(ps, borow[:, ocs], ones_r,
                                         start=False, stop=True)
                        nc.vector.tensor_tensor(xTs[oc][qb], xTs[oc][qb],
                                                ps, AL.add)

                # ---- ffn ----
                h2T = layer_norm(ln2g[l], ln2b[l])
                b2row = small.tile([1, D], BF16, tag="borow")
                nc.sync.dma_start(b2row, b2_s[l])
                for qb in range(NQB):
                    qs = slice(qb * QB, (qb + 1) * QB)
                    ffT = ffpool.tile([P, NF4T, QB], BF16, tag="fft")
                    for g in range(TPG):
                        b1_t = small.tile([P, NFC], F32, tag="b1")
                        nc.sync.dma_start(b1_t, b1_s[l, g])
                        for fh in range(4):
                            fs = slice(g * FFL + fh * (FFL // 4),
                                       g * FFL + (fh + 1) * (FFL // 4))
                            w1_t = w1p.tile([P, KT, FFL // 4], BF16, tag="w1")
                            nc.sync.dma_start(
                                w1_t, w1_s[l][:, fs].rearrange(
                                    "(t p) f -> p t f", p=P))
                            for fc in range(NFC // 4):
                                ps = psA.tile([P, QB], F32, tag="psA")
                                for kt in range(KT):
                                    nc.tensor.matmul(
                                        ps, w1_t[:, kt, fc * P:(fc + 1) * P],
                                        h2T[:, kt, qs],
                                        start=kt == 0, stop=kt == KT - 1)
                                fl = fh * (NFC // 4) + fc
                                fidx = g * NFC + fl
                                nc.scalar.activation(
                                    ffT[:, fidx, :], ps, ACT.Gelu,
                                    bias=b1_t[:, fl:fl + 1])
                    for oc in range(KT):
                        ocs = slice(oc * P, (oc + 1) * P)
                        w2_t = w2p.tile([P, NF4T, P], BF16, tag="w2")
                        nc.sync.dma_start(
                            w2_t, w2_s[l][:, ocs].rearrange(
                                "(t p) f -> p t f", p=P))
                        ps = psA.tile([P, QB], F32, tag="psA")
                        for f4t in range(NF4T):
                            nc.tensor.matmul(
                                ps, w2_t[:, f4t, :], ffT[:, f4t, :],
                                start=f4t == 0, stop=False)
                        nc.tensor.matmul(ps, b2row[:, ocs], ones_r,
                                         start=False, stop=True)
                        nc.vector.tensor_tensor(xTs[oc][qb], xTs[oc][qb],
                                                ps, AL.add)

            # ---- final LN + vocab-sharded head ----
            xfT = layer_norm(fng, fnb)
            for vc in range(NVC):
                vs = slice(vc * VC, (vc + 1) * VC)
                hw_t = qkpool.tile([P, KT, VC], BF16, tag="qk")
                nc.sync.dma_start(hw_t, hw_s[:, vs].rearrange("(t p) v -> p t v", p=P))
                hb_rf = rows.tile([1, VC], F32, tag="row")
                nc.sync.dma_start(hb_rf, hb_s[:, vs])
                hb_row = rows.tile([1, VC], F32, tag="row")
                nc.vector.tensor_copy(hb_row, hb_rf)
                hb_b = bcpool.tile([P, VC], F32, tag="bc")
                nc.gpsimd.partition_broadcast(hb_b, hb_row)
                for tc_ in range(NKT):
                    ps = psA.tile([P, QB], F32, tag="psA")
                    for kt in range(KT):
                        nc.tensor.matmul(
                            ps, xfT[:, kt, tc_ * P:(tc_ + 1) * P],
                            hw_t[:, kt, :], start=kt == 0, stop=kt == KT - 1)
                    lg = tmp.tile([P, VC], F32, tag="acc")
                    nc.vector.tensor_tensor(lg, ps, hb_b, AL.add)
                    nc.sync.dma_start(logits[tc_ * P:(tc_ + 1) * P, vs], lg)

    nc.finalize()
    return nc


def _prep_inputs(inputs):
    f = np.ascontiguousarray
    tokens = np.asarray(inputs["tokens"])
    tok_emb = np.asarray(inputs["tok_emb"], np.float32)
    pos_emb = np.asarray(inputs["pos_emb"], np.float32)

    Lb = L_BODY

    def bf(a):
        return f(np.asarray(a, np.float32).astype(BF_NP))

    def colmajor(a):  # [..., D] -> [..., P, KT] per-partition columns
        return f(a.reshape(*a.shape[:-1], KT, P).swapaxes(-1, -2).astype(np.float32))

    # multiplicative causal masks for the diagonal key tiles, bf16
    masks = (np.arange(P)[:, None] + 384
             <= np.arange(896)[None, :]).astype(BF_NP)

    wv = np.asarray(inputs["wv"], np.float32)[:Lb]          # [L, D, D]
    bv = np.asarray(inputs["bv"], np.float32)[:Lb]          # [L, D]
    # insert ones-columns: per head h, cols h*65..h*65+63 = V, col h*65+64 pad
    wv_p = np.zeros((Lb, D, H * (DH + 1)), np.float32)
    bv_p = np.zeros((Lb, 1, H * (DH + 1)), np.float32)
    for h in range(H):
        wv_p[:, :, h * (DH + 1):h * (DH + 1) + DH] = wv[:, :, h * DH:(h + 1) * DH]
        bv_p[:, 0, h * (DH + 1):h * (DH + 1) + DH] = bv[:, h * DH:(h + 1) * DH]
        bv_p[:, 0, h * (DH + 1) + DH] = 1.0

    b1 = np.asarray(inputs["b1"], np.float32)[:Lb]

    base = {
        "masks": masks,
        "onesc": np.ones((P, 2), np.float32),
        "onescb": np.ones((P, 2), BF_NP),
        "onesr": np.ones((1, QB), BF_NP),
        "wq_s": bf(np.asarray(inputs["wq"], np.float32)[:Lb] * SCALE),
        "wk_s": bf(np.asarray(inputs["wk"], np.float32)[:Lb]),
        "wv_s": bf(wv_p),
        "wo_s": bf(np.asarray(inputs["wo"], np.float32)[:Lb]),
        "w1_s": bf(np.asarray(inputs["w1"], np.float32)[:Lb]),
        "w2_s": bf(np.asarray(inputs["w2"], np.float32)[:Lb]),
        "ln1g": colmajor(np.asarray(inputs["ln1_g"], np.float32)[:Lb]),
        "ln1b": colmajor(np.asarray(inputs["ln1_b"], np.float32)[:Lb]),
        "ln2g": colmajor(np.asarray(inputs["ln2_g"], np.float32)[:Lb]),
        "ln2b": colmajor(np.asarray(inputs["ln2_b"], np.float32)[:Lb]),
        "bq_s": bf((np.asarray(inputs["bq"], np.float32)[:Lb] * SCALE)[:, None, :]),
        "bk_s": bf(np.asarray(inputs["bk"], np.float32)[:Lb][:, None, :]),
        "bv_s": bf(bv_p),
        "bo_s": bf(np.asarray(inputs["bo"], np.float32)[:Lb][:, None, :]),
        "b1_s": f(b1.reshape(Lb, TPG, NFC, P).swapaxes(-1, -2).astype(np.float32)),
        "b2_s": bf(np.asarray(inputs["b2"], np.float32)[:Lb][:, None, :]),
        "fng": colmajor(np.asarray(inputs["fn_g"], np.float32)),
        "fnb": colmajor(np.asarray(inputs["fn_b"], np.float32)),
    }

    head_w = np.asarray(inputs["head_w"], np.float32)
    head_b = np.asarray(inputs["head_b"], np.float32)

    in_maps = []
    for c in range(NCORES):
        b = c // 4
        gsl = c % 4
        v0, vn = VSTART[gsl], VSLICE[gsl]
        hw_pad = np.zeros((D, VPAD), BF_NP)
        hw_pad[:, :vn] = head_w[:, v0:v0 + vn].astype(BF_NP)
        hb_pad = np.zeros((1, VPAD), np.float32)
        hb_pad[0, :vn] = head_b[v0:v0 + vn]
        x0 = tok_emb[tokens[b]] + pos_emb[:S]
        m = {"x0T": f(x0.T.astype(np.float32)), "hw_s": hw_pad, "hb_s": hb_pad}
        m.update(base)
        in_maps.append(m)
    return in_maps


def _get_nc():
    key = ("nc", L_BODY)
    if key not in _CACHE:
        _CACHE[key] = _build()
    return _CACHE[key]


def kernel(**inputs):
    nc = _get_nc()
    in_maps = _prep_inputs(inputs)
    res = bass_utils.run_bass_kernel_spmd(nc, in_maps, core_ids=list(range(NCORES)))
    out = np.empty((B, S, V), np.float32)
    for c in range(NCORES):
        b, gsl = c // 4, c % 4
        v0, vn = VSTART[gsl], VSLICE[gsl]
        out[b, :, v0:v0 + vn] = res.results[c]["logits"][:, :vn]
    return out
